# revision 24
# baseline (speedup 1.0000x reference)
"""GAT (decomposed-attention) Bass kernel for 8 Trainium2 NeuronCores — v3.

Strategy: destination-sharded edge processing with batched dma_gather.
- Host: shard edges by dst node (12500/core), order chunks bucket-major
  (4 src-row ranges of 32768 for int16 dma_gather indices), window-minor;
  equalize per-(bucket,window) chunk counts across cores (SPMD); ship
  one-hot sel/selT matrices as fp8 and gather indices as int16.
- Device: phase P projects vert @ [W | W.a_src | W.a_dst] per 128-node
  window, stores [g | e_s] rows (256B) in a bf16 table + AllGather;
  e_d stays in SBUF (edstage). Phase E per 16-chunk group: one dma_gather
  of 2048 src rows, e_d per edge via fp8 selT matmuls into one PSUM tile,
  scores = gathered e_s + e_d, ex = max(exp(s), exp(0.2 s)), rhs =
  [g*ex | ex] bf16, per-chunk fp8 sel matmul segment-sum into per-window
  PSUM runs accumulated into U in SBUF. Phase O: out = elu(U/denom).
"""
import os
import sys
import types

sys.path.insert(0, '/opt/trn_rl_repo')
sys.path.insert(0, '/opt/trn_rl_repo/concourse')

import numpy as np
import ml_dtypes

import concourse.bass as bass
import concourse.bacc as bacc
import concourse.mybir as mybir
import concourse.tile as tile
from concourse.bass_utils import run_bass_kernel_spmd
from concourse.masks import make_identity

F32 = mybir.dt.float32
BF16 = mybir.dt.bfloat16
I16 = mybir.dt.int16
F8 = mybir.dt.float8e4

N_CORES = 8
N_NODES = 100000
N_EDGES = 1600000
IN_F = 128
N_HEADS = 8
HEAD_D = 8
HD = N_HEADS * HEAD_D          # 64
NEG_SLOPE = 0.2
NPC = N_NODES // N_CORES       # 12500
NPP = 12544                    # padded to 128*98
NWIN = NPP // 128              # 98
NROWS = N_CORES * NPP          # 100352 table rows
BUCKET = 32768
NBUCK = (NROWS + BUCKET - 1) // BUCKET   # 4
GC = 32                        # chunks per gather group
NQ = 4                         # SWDGE queues (parallel Q7 descriptor gen)

LAST_EXEC_NS = None


def _install_ntff_shim():
    try:
        _HOOK = [None]
        mod = types.ModuleType("antenv.axon_hooks")
        mod.set_axon_ntff_profile_hook = lambda h: _HOOK.__setitem__(0, h)
        mod.get_axon_ntff_profile_hook = lambda: _HOOK[0]
        sys.modules.setdefault("antenv.axon_hooks", mod)
        import antenv
        if not hasattr(antenv, "axon_hooks"):
            antenv.axon_hooks = sys.modules["antenv.axon_hooks"]
        from trn_agent_boot.trn_boot import _ntff_profile_via_ctypes
        hook = _ntff_profile_via_ctypes('/opt/axon/libaxon_pjrt.so')
        sys.modules["antenv.axon_hooks"].set_axon_ntff_profile_hook(hook)
        return hook is not None
    except Exception:
        return False


QROWS = 32 * NWIN              # 3136 rows per (core, quarter)
TQROWS = N_CORES * QROWS       # 25088 rows per quarter-table (int16 ok)


def _grow(idx):
    """Global node index -> (quarter, row within quarter-table)."""
    c, l = idx // NPC, idx % NPC
    p = l % 128
    r = p // 32
    row = c * QROWS + (p - 32 * r) * NWIN + l // 128
    return r.astype(np.int64), row.astype(np.int64)


def _prep_host(vert, edge, W, a_src, a_dst):
    src = np.asarray(edge[0], np.int64)
    dst = np.asarray(edge[1], np.int64)
    order = np.argsort(dst, kind="stable")
    s_src = src[order]
    s_dst = dst[order]
    rq_all, srow_all = _grow(s_src)

    core_lo = [np.searchsorted(s_dst, c * NPC) for c in range(N_CORES)]
    core_lo.append(len(s_dst))

    # per-core edge fields, sorted by (bucket, window) stable
    per_core = []
    cnt = np.zeros((N_CORES, NBUCK, NWIN), np.int64)
    for c in range(N_CORES):
        lo, hi = core_lo[c], core_lo[c + 1]
        ldst = s_dst[lo:hi] - c * NPC
        w = ldst // 128
        dloc = ldst % 128
        srow = srow_all[lo:hi]
        r = rq_all[lo:hi]
        key = r * NWIN + w
        o2 = np.argsort(key, kind="stable")
        per_core.append((r[o2], w[o2], dloc[o2], srow[o2]))
        np.add.at(cnt[c], (r[o2], w[o2]), 1)

    # shared chunk counts per (bucket, window); bucket totals GC-aligned
    k = np.ceil(cnt.max(axis=0) / 128.0).astype(np.int64)      # [NBUCK, NWIN]
    for r in range(NBUCK):
        tot = int(k[r].sum())
        k[r, NWIN - 1] += (-tot) % GC
    nch = int(k.sum())

    # chunk layout: bucket-major, window-minor; run = (r, w) chunk span
    ch_base = np.zeros((NBUCK, NWIN), np.int64)
    runs = []          # (r, w, c0, c1)
    pos = 0
    for r in range(NBUCK):
        for w in range(NWIN):
            ch_base[r, w] = pos
            if k[r, w] > 0:
                runs.append((r, w, pos, pos + int(k[r, w])))
            pos += int(k[r, w])
    grp_bucket = []    # bucket of each GC-group (groups never straddle buckets)
    for r in range(NBUCK):
        nb = int(k[r].sum())
        grp_bucket += [r] * (nb // GC)

    w_of = np.zeros(nch, np.int64)
    for (r, w, c0, c1) in runs:
        w_of[c0:c1] = w

    # weight folding
    Wf = np.asarray(W, np.float32).reshape(IN_F, HD)
    W_s = np.einsum("fhd,hd->fh", np.asarray(W, np.float32), np.asarray(a_src, np.float32))
    W_d = np.einsum("fhd,hd->fh", np.asarray(W, np.float32), np.asarray(a_dst, np.float32))
    W_ext = np.concatenate([Wf, W_s, W_d], axis=1).astype(np.float32)   # [128, 80]

    vert_np = np.asarray(vert, np.float32)
    in_maps = []
    for c in range(N_CORES):
        r, w, dloc, lrow = per_core[c]
        n = len(r)
        # slot position within the (r, w) run for each edge
        key = r * NWIN + w
        # edges are sorted by key; position within run = index - first index of key
        first = np.searchsorted(key, key)   # first occurrence of each key value
        posn = np.arange(n) - first
        chv = ch_base[r, w] + posn // 128
        pv = posn % 128

        # gather-unit = 8 chunks (1024 idxs per dma_gather call)
        idx16 = np.zeros((16, nch * 8), np.int16)
        jj = chv % 8 * 128 + pv             # j within gather-unit
        gg = chv // 8
        idx16[jj % 16, gg * 64 + jj // 16] = lrow.astype(np.int16)
        idx16 = np.tile(idx16, (8, 1))

        sel = np.zeros((128, nch, 128), ml_dtypes.float8_e4m3fn)
        sel[pv, chv, dloc] = 1.0
        selT = np.zeros((128, nch, 128), ml_dtypes.float8_e4m3fn)
        selT[dloc, chv, pv] = 1.0

        vs = np.zeros((NPP, IN_F), np.float32)
        vs[:NPC] = vert_np[c * NPC:(c + 1) * NPC]
        in_maps.append({
            "vert_shard": vs,
            "W_ext": W_ext,
            "idx16": idx16,
            "sel8": sel.reshape(128, nch * 128).view(np.float32),
            "selT8": selT.reshape(128, nch * 128).view(np.float32),
        })
    return in_maps, nch, runs, grp_bucket, w_of.tolist()


def _build(nch, runs, grp_bucket, w_of):
    nc = bacc.Bacc("TRN2", target_bir_lowering=False, debug=False,
                   num_devices=N_CORES, num_swdge_queues=NQ)
    vert_shard = nc.dram_tensor("vert_shard", [NPP, IN_F], F32, kind="ExternalInput")
    W_ext = nc.dram_tensor("W_ext", [IN_F, 80], F32, kind="ExternalInput")
    idx16 = nc.dram_tensor("idx16", [128, nch * 8], I16, kind="ExternalInput")
    sel8 = nc.dram_tensor("sel8", [128, nch * 32], F32, kind="ExternalInput")
    selT8 = nc.dram_tensor("selT8", [128, nch * 32], F32, kind="ExternalInput")
    out = nc.dram_tensor("out", [128, NWIN * HD], F32, kind="ExternalOutput")

    t1_local = [nc.dram_tensor(f"t1_local{q}", [QROWS, 64], F32)
                for q in range(NBUCK)]
    T1q = [nc.dram_tensor(f"T1q{q}", [TQROWS, 64], F32, addr_space="Shared")
           for q in range(NBUCK)]

    rg = [list(range(N_CORES))]
    ngrp = nch // GC

    with tile.TileContext(nc) as tc:
        _glob_cm = tc.tile_pool(name="glob", bufs=1)
        glob = _glob_cm.__enter__()
        identf = glob.tile([128, 128], F32)
        make_identity(nc, identf[:])
        edstage = glob.tile([128, NWIN * N_HEADS], BF16)
        U = glob.tile([128, NWIN * 72], F32)
        nc.vector.memset(U[:], 0.0)
        idx_sb = glob.tile([128, nch * 8], I16)
        nc.sync.dma_start(out=idx_sb[:], in_=idx16[:])

        # ---- phase P: projection + table + AllGather ----
        with tc.tile_pool(name="pres", bufs=1) as pres, \
             tc.tile_pool(name="pv", bufs=3) as pv, \
             tc.tile_pool(name="pps", bufs=3, space="PSUM") as pps:
            wext_sb = pres.tile([IN_F, 80], F32)
            nc.sync.dma_start(out=wext_sb[:], in_=W_ext[:])
            gstage = pres.tile([128, NWIN * 128], BF16)
            nc.vector.memset(gstage[:], 0.0)
            for t in range(NWIN):
                vtile = pv.tile([128, IN_F], F32, tag="vt")
                nc.sync.dma_start(out=vtile[:], in_=vert_shard[t * 128:(t + 1) * 128, :])
                ps_t = pps.tile([128, 128], F32, tag="pst")
                nc.tensor.transpose(out=ps_t[:], in_=vtile[:], identity=identf[:])
                vtT = pv.tile([128, 128], F32, tag="vtT")
                nc.vector.tensor_copy(out=vtT[:], in_=ps_t[:])
                ps_g = pps.tile([128, 80], F32, tag="psg")
                nc.tensor.matmul(out=ps_g[:], lhsT=vtT[:], rhs=wext_sb[:],
                                 start=True, stop=True)
                nc.vector.tensor_copy(out=gstage[:, t * 128:t * 128 + 72],
                                      in_=ps_g[:, 0:72])
                nc.vector.tensor_copy(out=edstage[:, t * 8:(t + 1) * 8],
                                      in_=ps_g[:, 72:80])
            for q in range(NBUCK):
                nc.sync.dma_start(
                    out=t1_local[q][:].bitcast(BF16)
                        .rearrange("(p w) d -> p w d", p=32),
                    in_=gstage[32 * q:32 * (q + 1), :]
                        .rearrange("p (w d) -> p w d", d=128))
            for q in range(NBUCK):
                nc.gpsimd.collective_compute(
                    "AllGather", mybir.AluOpType.bypass, replica_groups=rg,
                    ins=[t1_local[q][:]], outs=[T1q[q][:]])

        # ---- phase E ----
        T1b = [T1q[q][:].bitcast(BF16) for q in range(NBUCK)]
        with tc.tile_pool(name="pg", bufs=3) as pg, \
             tc.tile_pool(name="pped", bufs=2, space="PSUM") as pped, \
             tc.tile_pool(name="ppw", bufs=2, space="PSUM") as ppw:
            grp = {}
            self_qn = [0]

            def ensure_grp(g):
                if g in grp:
                    return grp[g]
                r = grp_bucket[g]
                lo = g * GC
                gat = pg.tile([128, GC * 128], BF16, tag="gat")
                for h in range(GC // 8):
                    nc.gpsimd.dma_gather(
                        out_ap=gat[:, h * 1024:(h + 1) * 1024]
                            .rearrange("p (c e) -> p c e", e=128),
                        in_ap=T1b[r],
                        idxs_ap=idx_sb[:, (g * (GC // 8) + h) * 64:
                                       (g * (GC // 8) + h + 1) * 64],
                        num_idxs=1024,
                        num_idxs_reg=1024,
                        elem_size=128,
                        queue_num=self_qn[0] % NQ,
                    )
                    self_qn[0] += 1
                selt_sb = pg.tile([128, GC * 32], F32, tag="selt")
                nc.sync.dma_start(out=selt_sb[:],
                                  in_=selT8[:, lo * 32:(lo + GC) * 32])
                sel_sb = pg.tile([128, GC * 32], F32, tag="sel")
                nc.sync.dma_start(out=sel_sb[:],
                                  in_=sel8[:, lo * 32:(lo + GC) * 32])
                seltf8 = selt_sb[:].bitcast(F8)
                ps_ed = pped.tile([128, GC * 8], F32, tag="psed")
                for c in range(GC):
                    w = w_of[lo + c]
                    nc.tensor.matmul(
                        out=ps_ed[:, c * 8:(c + 1) * 8],
                        lhsT=seltf8[:, c * 128:(c + 1) * 128],
                        rhs=edstage[:, w * 8:(w + 1) * 8],
                        start=True, stop=True)
                gat3 = gat[:].rearrange("p (c e) -> p c e", e=128)
                sco = pg.tile([128, GC * 8], F32, tag="sco")
                nc.vector.tensor_tensor(
                    out=sco[:].rearrange("p (c k) -> p c k", k=8),
                    in0=gat3[:, :, 64:72],
                    in1=ps_ed[:].rearrange("p (c k) -> p c k", k=8),
                    op=mybir.AluOpType.add)
                e1 = pg.tile([128, GC * 8], F32, tag="e1")
                nc.scalar.activation(e1[:], sco[:], mybir.ActivationFunctionType.Exp)
                e2 = pg.tile([128, GC * 8], F32, tag="e2")
                nc.scalar.activation(e2[:], sco[:], mybir.ActivationFunctionType.Exp,
                                     scale=NEG_SLOPE)
                rhs = pg.tile([128, GC * 72], BF16, tag="rhs")
                rhs3 = rhs[:].rearrange("p (c k) -> p c k", k=72)
                nc.vector.tensor_tensor(
                    out=rhs3[:, :, 64:72],
                    in0=e1[:].rearrange("p (c k) -> p c k", k=8),
                    in1=e2[:].rearrange("p (c k) -> p c k", k=8),
                    op=mybir.AluOpType.max)
                nc.vector.tensor_tensor(
                    out=rhs3[:, :, 0:64].rearrange("p c (h d) -> p c h d", d=HEAD_D),
                    in0=gat3[:, :, 0:64].rearrange("p c (h d) -> p c h d", d=HEAD_D),
                    in1=rhs3[:, :, 64:72].rearrange("p c (h o) -> p c h o", o=1)
                        .to_broadcast([128, GC, N_HEADS, HEAD_D]),
                    op=mybir.AluOpType.mult)
                grp[g] = (sel_sb, rhs)
                grp.pop(g - 2, None)
                return grp[g]

            for (r, w, c0, c1) in runs:
                psw = ppw.tile([128, 72], F32, tag="psw")
                for ch in range(c0, c1):
                    sel_sb, rhs = ensure_grp(ch // GC)
                    cc = ch % GC
                    nc.tensor.matmul(
                        out=psw[:],
                        lhsT=sel_sb[:].bitcast(F8)[:, cc * 128:(cc + 1) * 128],
                        rhs=rhs[:, cc * 72:(cc + 1) * 72],
                        start=(ch == c0), stop=(ch == c1 - 1))
                nc.vector.tensor_tensor(
                    out=U[:, w * 72:(w + 1) * 72],
                    in0=U[:, w * 72:(w + 1) * 72],
                    in1=psw[:], op=mybir.AluOpType.add)

            # ---- phase O ----
            U3 = U[:].rearrange("p (w k) -> p w k", k=72)
            with tc.tile_pool(name="po", bufs=2) as po, \
                 tc.tile_pool(name="pden", bufs=1) as pden:
                den = pden.tile([128, NWIN * N_HEADS], F32)
                nc.vector.tensor_scalar_max(
                    den[:].rearrange("p (w k) -> p w k", k=N_HEADS),
                    U3[:, :, 64:72], 1e-16)
                rec = pden.tile([128, NWIN * N_HEADS], F32)
                nc.vector.reciprocal(rec[:], den[:])
                WB = 14
                for b in range(0, NWIN, WB):
                    nb = min(WB, NWIN - b)
                    agg = po.tile([128, WB * HD], F32, tag="agg")
                    nc.vector.tensor_tensor(
                        out=agg[:, :nb * HD].rearrange("p (w h d) -> p w h d",
                                                       h=N_HEADS, d=HEAD_D),
                        in0=U3[:, b:b + nb, 0:HD]
                            .rearrange("p w (h d) -> p w h d", d=HEAD_D),
                        in1=rec[:, b * N_HEADS:(b + nb) * N_HEADS]
                            .rearrange("p (w h) -> p w h", h=N_HEADS)
                            .rearrange("p w (h o) -> p w h o", o=1)
                            .to_broadcast([128, nb, N_HEADS, HEAD_D]),
                        op=mybir.AluOpType.mult)
                    tmin = po.tile([128, WB * HD], F32, tag="tmin")
                    nc.vector.tensor_scalar_min(tmin[:, :nb * HD], agg[:, :nb * HD], 0.0)
                    texp = po.tile([128, WB * HD], F32, tag="texp")
                    nc.scalar.activation(texp[:, :nb * HD], tmin[:, :nb * HD],
                                         mybir.ActivationFunctionType.Exp)
                    tpos = po.tile([128, WB * HD], F32, tag="tpos")
                    nc.vector.tensor_scalar_max(tpos[:, :nb * HD], agg[:, :nb * HD], 0.0)
                    tres = po.tile([128, WB * HD], F32, tag="tres")
                    nc.vector.tensor_tensor(out=tres[:, :nb * HD], in0=texp[:, :nb * HD],
                                            in1=tpos[:, :nb * HD], op=mybir.AluOpType.add)
                    nc.vector.tensor_scalar_add(tres[:, :nb * HD], tres[:, :nb * HD], -1.0)
                    nc.sync.dma_start(out=out[:, b * HD:(b + nb) * HD],
                                      in_=tres[:, :nb * HD])
        _glob_cm.__exit__(None, None, None)

    nc.compile()
    return nc


def kernel(vert, edge, W, a_src, a_dst):
    global LAST_EXEC_NS
    in_maps, nch, runs, grp_bucket, w_of = _prep_host(vert, edge, W, a_src, a_dst)
    nc = _build(nch, runs, grp_bucket, w_of)
    trace = os.environ.get("GAT_TRACE", "1") == "1" and _install_ntff_shim()
    try:
        res = run_bass_kernel_spmd(nc, in_maps, core_ids=list(range(N_CORES)),
                                   trace=trace)
    except Exception:
        if not trace:
            raise
        res = run_bass_kernel_spmd(nc, in_maps, core_ids=list(range(N_CORES)),
                                   trace=False)
    LAST_EXEC_NS = res.exec_time_ns
    outs = []
    for c in range(N_CORES):
        o = np.asarray(res.results[c]["out"]).reshape(128, NWIN, HD)
        o = o.transpose(1, 0, 2).reshape(NPP, HD)[:NPC]
        outs.append(o)
    return np.concatenate(outs, axis=0).astype(np.float32)
